# revision 8
# baseline (speedup 1.0000x reference)
"""Trainium2 Bass kernel for nn_AttentionCell (additive attention + LSTMCell step).

Data-parallel over 8 NeuronCores: batch B=256 is split into 8 shards of 32;
all parameters are replicated. Each core computes, for its 32 batches:

    proj  = batch_H @ Wi2h^T                      [32, 512, 512]
    p     = h @ Wh2h^T + bh2h                     [32, 512]
    e     = tanh(proj + p[:, None, :]) @ Wscore^T [32, 512]
    alpha = softmax(e, axis=t)                    [32, 512]
    ctx   = sum_t alpha * batch_H                 [32, 512]
    x     = [ctx, char_onehots]                   [32, 608]
    gates = x @ W_ih^T + b_ih + h @ W_hh^T + b_hh [32, 2048]
    i,f,g,o -> c_new = sig(f)*c + sig(i)*tanh(g); h_new = sig(o)*tanh(c_new)

Layout strategy: batch_H is DMA'd in natural [t, d] layout with f32->bf16
cast-on-DMA (SWDGE), kept resident for the context matvec, and xbar
DMA-transposed (bf16) into [d, t] tiles for the projection matmul. The h2h
bias rides the ScalarE tanh as a per-partition bias. All weight transposes
also go through the xbar. The big matmuls run in bf16 (fp32 PSUM accum).
"""

import sys

if "/opt/trn_rl_repo" not in sys.path:
    sys.path.insert(0, "/opt/trn_rl_repo")

import numpy as np

import concourse.bass as bass
import concourse.tile as tile
from concourse import bacc, mybir
from concourse.bass_utils import run_bass_kernel_spmd

F32 = mybir.dt.float32
BF16 = mybir.dt.bfloat16
AF = mybir.ActivationFunctionType
AX = mybir.AxisListType

B, T, D, H, NE = 256, 512, 512, 512, 96
NCORES = 8
BC = B // NCORES  # 32 batches per core
P = 128
ND, NH, NT = D // P, H // P, T // P  # 4, 4, 4
GB = 8              # batches per softmax group
NGRP = BC // GB     # 4 groups
DX = 640            # padded x width: 512 ctx + 96 onehots + 32 zero pad
NX = DX // P        # 5 contraction chunks for W_ih
G4 = 4              # four 512-wide gate blocks (i, f, g, o)

_NC_CACHE = None
_run_kwargs = {}
_last_result = [None]


def build_nc():
    nc = bacc.Bacc()

    # ---- per-core I/O ----
    d_h = nc.declare_dram_parameter("h", [BC, H], F32, isOutput=False)
    d_c = nc.declare_dram_parameter("c", [BC, H], F32, isOutput=False)
    d_bh = nc.declare_dram_parameter("batch_H", [BC, T, D], F32, isOutput=False)
    d_oh = nc.declare_dram_parameter("char_onehots", [BC, NE], F32, isOutput=False)
    d_wi2h = nc.declare_dram_parameter("Wi2h", [H, D], F32, isOutput=False)
    d_wh2h = nc.declare_dram_parameter("Wh2h", [H, H], F32, isOutput=False)
    d_bh2h = nc.declare_dram_parameter("bh2h", [1, H], F32, isOutput=False)
    d_wsc = nc.declare_dram_parameter("Wscore", [1, H], F32, isOutput=False)
    d_wih = nc.declare_dram_parameter("W_ih", [4 * H, D + NE], F32, isOutput=False)
    d_whh = nc.declare_dram_parameter("W_hh", [4 * H, H], F32, isOutput=False)
    d_bih = nc.declare_dram_parameter("b_ih", [1, 4 * H], F32, isOutput=False)
    d_bhh = nc.declare_dram_parameter("b_hh", [1, 4 * H], F32, isOutput=False)
    d_hout = nc.declare_dram_parameter("h_out", [BC, H], F32, isOutput=True)
    d_cout = nc.declare_dram_parameter("c_out", [BC, H], F32, isOutput=True)
    d_aout = nc.declare_dram_parameter("alpha_out", [BC, T], F32, isOutput=True)

    from contextlib import ExitStack

    with tile.TileContext(nc) as tc, ExitStack() as ctx:
        const = ctx.enter_context(tc.tile_pool(name="const", bufs=1))
        prep = ctx.enter_context(tc.tile_pool(name="prep", bufs=2))
        lstm = ctx.enter_context(tc.tile_pool(name="lstm", bufs=1))
        stag = ctx.enter_context(tc.tile_pool(name="stag", bufs=1))
        xbp = ctx.enter_context(tc.tile_pool(name="xbp", bufs=10))
        xtp = ctx.enter_context(tc.tile_pool(name="xtp", bufs=2))
        thp = ctx.enter_context(tc.tile_pool(name="thp", bufs=2))
        grp = ctx.enter_context(tc.tile_pool(name="grp", bufs=1))
        ps_prep = ctx.enter_context(tc.tile_pool(name="ps_prep", bufs=2, space="PSUM"))
        ps_proj = ctx.enter_context(tc.tile_pool(name="ps_proj", bufs=2, space="PSUM"))
        ps_vec = ctx.enter_context(tc.tile_pool(name="ps_vec", bufs=2, space="PSUM"))
        ps_g = ctx.enter_context(tc.tile_pool(name="ps_g", bufs=2, space="PSUM"))

        if True:
            # ============ constant prep ============
            ones_f = const.tile([1, BC], F32)
            nc.vector.memset(ones_f, 1.0)

            # Wi2h^T (bf16): WT[p, j, h] = Wi2h[h, d=4p+j]
            WT = const.tile([P, ND, H], BF16)
            for ht in range(NH):
                wn = prep.tile([P, D], BF16, tag="wn")
                nc.gpsimd.dma_start(out=wn, in_=d_wi2h[ht * P : (ht + 1) * P, :])
                nc.sync.dma_start(
                    out=WT[:, :, ht * P : (ht + 1) * P], in_=wn, transpose=True
                )

            # Wh2h^T (bf16): WhT[p, j, h'] = Wh2h[h', hh=4p+j]
            WhT = const.tile([P, NH, H], BF16)
            for ht in range(NH):
                wn = prep.tile([P, H], BF16, tag="wn")
                nc.gpsimd.dma_start(out=wn, in_=d_wh2h[ht * P : (ht + 1) * P, :])
                nc.sync.dma_start(
                    out=WhT[:, :, ht * P : (ht + 1) * P], in_=wn, transpose=True
                )

            # W_ih^T (bf16, padded to DX=640 rows): WihT[p, j, m] = W_ih[m, d'=5p+j]
            WihT = const.tile([P, NX, 4 * H], BF16)
            for ht in range(4 * H // P):
                wn = prep.tile([P, DX], BF16, tag="wn2")
                nc.vector.memset(wn, 0.0)
                nc.gpsimd.dma_start(
                    out=wn[:, : D + NE], in_=d_wih[ht * P : (ht + 1) * P, :]
                )
                nc.sync.dma_start(
                    out=WihT[:, :, ht * P : (ht + 1) * P], in_=wn, transpose=True
                )

            # W_hh^T (bf16): WhhT[p, j, m] = W_hh[m, hh=4p+j]
            WhhT = const.tile([P, NH, 4 * H], BF16)
            for ht in range(4 * H // P):
                wn = prep.tile([P, H], BF16, tag="wn")
                nc.gpsimd.dma_start(out=wn, in_=d_whh[ht * P : (ht + 1) * P, :])
                nc.sync.dma_start(
                    out=WhhT[:, :, ht * P : (ht + 1) * P], in_=wn, transpose=True
                )

            # h: natural f32, bf16 copy, transposed hT[p, j, b] = h[b, hh=4p+j]
            hn_f = prep.tile([BC, H], F32, tag="ldh")
            nc.sync.dma_start(out=hn_f, in_=d_h[:, :])
            hn_bf = prep.tile([BC, H], BF16, tag="ldhb")
            nc.vector.tensor_copy(hn_bf, hn_f)
            hT = const.tile([P, NH, BC], BF16)
            nc.sync.dma_start(out=hT, in_=hn_bf, transpose=True)

            # bh2h^T chunks (f32) via K=1 matmul against ones
            bh2h_sb = prep.tile([1, H], F32, tag="ldsm")
            nc.sync.dma_start(out=bh2h_sb, in_=d_bh2h[:, :])
            bhT = const.tile([P, NH, 1], F32)
            for ci in range(NH):
                pst = ps_prep.tile([P, 1], F32, tag="prep")
                nc.tensor.matmul(
                    pst, bh2h_sb[:, ci * P : (ci + 1) * P], ones_f[:, 0:1]
                )
                nc.scalar.activation(bhT[:, ci, :], pst, AF.Copy)

            # Wscore^T chunks (bf16) via K=1 matmul
            wsc_sb = prep.tile([1, H], F32, tag="ldsm2")
            nc.sync.dma_start(out=wsc_sb, in_=d_wsc[:, :])
            wT = const.tile([P, NH, 1], BF16)
            for ci in range(NH):
                pst = ps_prep.tile([P, 1], F32, tag="prep")
                nc.tensor.matmul(pst, wsc_sb[:, ci * P : (ci + 1) * P], ones_f[:, 0:1])
                nc.vector.tensor_copy(wT[:, ci, :], pst)

            # prev_hidden_proj^T: pT[:, c', b] = (h @ Wh2h^T + bh2h)[b, h'=128c'+p]
            pT = const.tile([P, NH, BC], F32)
            for ci in range(NH):
                psp = ps_prep.tile([P, BC], F32, tag="prep")
                for j in range(NH):
                    nc.tensor.matmul(
                        psp,
                        WhT[:, j, ci * P : (ci + 1) * P],
                        hT[:, j, :],
                        start=(j == 0),
                        stop=(j == NH - 1),
                    )
                nc.scalar.activation(pT[:, ci, :], psp, AF.Identity, bias=bhT[:, ci, :])

            # LSTM bias sum (f32) for the K=1 broadcast matmul
            bih_sb = prep.tile([1, 4 * H], F32, tag="ldb1")
            nc.sync.dma_start(out=bih_sb, in_=d_bih[:, :])
            bhh_sb = prep.tile([1, 4 * H], F32, tag="ldb2")
            nc.sync.dma_start(out=bhh_sb, in_=d_bhh[:, :])
            bsum = const.tile([1, 4 * H], F32)
            nc.vector.tensor_add(bsum, bih_sb, bhh_sb)

            # x staging [ctx | onehots | 0-pad] in bf16, filled later
            x_sb = const.tile([BC, DX], BF16)
            nc.vector.memset(x_sb[:, D:], 0.0)
            oh_f = prep.tile([BC, NE], F32, tag="ldoh")
            nc.sync.dma_start(out=oh_f, in_=d_oh[:, :])
            nc.vector.tensor_copy(x_sb[:, D : D + NE], oh_f)

            cn_f = lstm.tile([BC, H], F32, tag="cn")
            nc.sync.dma_start(out=cn_f, in_=d_c[:, :])

            # ============ main loop: groups of 8 batches ============
            xb_tiles = {}
            for g in range(NGRP):
                es = stag.tile([1, GB, T], BF16, tag="es")
                for bi in range(GB):
                    b = g * GB + bi
                    # natural bf16 load: Xb[p, tcb, d] = batch_H[b, t=128*tcb+p, d]
                    xb = xbp.tile([P, NT, D], BF16, tag="xb")
                    xb_tiles[b] = xb
                    nc.gpsimd.dma_start(
                        out=xb,
                        in_=d_bh[b].rearrange("(n p) d -> p n d", p=P),
                    )
                    # transpose to XT[p, j, t] = X[t, d=4p+j]
                    xt = xtp.tile([P, ND, T], BF16, tag="xt")
                    for tcb in range(NT):
                        nc.sync.dma_start(
                            out=xt[:, :, tcb * P : (tcb + 1) * P],
                            in_=xb[:, tcb, :],
                            transpose=True,
                        )
                    # proj + tanh(. + p_b) per h-tile
                    th = thp.tile([P, NH, T], BF16, tag="th")
                    for ht in range(NH):
                        psy = ps_proj.tile([P, T], F32, tag="proj")
                        for j in range(ND):
                            nc.tensor.matmul(
                                psy,
                                WT[:, j, ht * P : (ht + 1) * P],
                                xt[:, j, :],
                                start=(j == 0),
                                stop=(j == ND - 1),
                            )
                        nc.scalar.activation(
                            th[:, ht, :], psy, AF.Tanh, bias=pT[:, ht, b : b + 1]
                        )
                    # e[b, t] = sum_h Wscore[h] * tanh(...)
                    pse = ps_vec.tile([1, T], F32, tag="vec")
                    for ht in range(NH):
                        nc.tensor.matmul(
                            pse,
                            wT[:, ht, :],
                            th[:, ht, :],
                            start=(ht == 0),
                            stop=(ht == NH - 1),
                        )
                    nc.scalar.activation(es[:, bi, :], pse, AF.Copy)

                # gather the group's e rows onto 8 partitions
                E = grp.tile([GB, T], BF16, tag="E")
                nc.sync.dma_start(out=E, in_=es)
                # softmax over t (free dim)
                mx = grp.tile([GB, 1], F32, tag="mx")
                nc.vector.reduce_max(mx, E, axis=AX.X)
                nmx = grp.tile([GB, 1], F32, tag="nmx")
                nc.vector.tensor_scalar_mul(nmx, mx, -1.0)
                ex = grp.tile([GB, T], F32, tag="ex")
                ssum = grp.tile([GB, 1], F32, tag="ssum")
                nc.scalar.activation(ex, E, AF.Exp, bias=nmx, accum_out=ssum)
                rs = grp.tile([GB, 1], F32, tag="rs")
                nc.vector.reciprocal(rs, ssum)
                alpha = grp.tile([GB, T], F32, tag="alpha")
                nc.vector.tensor_scalar_mul(alpha, ex, rs)
                nc.sync.dma_start(
                    out=d_aout[g * GB : (g + 1) * GB, :], in_=alpha
                )
                # alpha^T tiles (bf16, t-blocked to match Xb's layout)
                alpha_bf = grp.tile([2 * GB, T], BF16, tag="alpha_bf")
                nc.vector.memset(alpha_bf, 0.0)
                nc.vector.tensor_copy(alpha_bf[:GB, :], alpha)
                aT = grp.tile([P, NT, 2 * GB], BF16, tag="aT")
                for tcb in range(NT):
                    nc.sync.dma_start(
                        out=aT[:, tcb, :],
                        in_=alpha_bf[:, tcb * P : (tcb + 1) * P],
                        transpose=True,
                    )
                # context rows
                cs = stag.tile([1, GB, D], BF16, tag="cs")
                for bi in range(GB):
                    b = g * GB + bi
                    psc = ps_vec.tile([1, D], F32, tag="vec")
                    for tcb in range(NT):
                        nc.tensor.matmul(
                            psc,
                            aT[:, tcb, bi : bi + 1],
                            xb_tiles[b][:, tcb, :],
                            start=(tcb == 0),
                            stop=(tcb == NT - 1),
                        )
                    nc.scalar.activation(cs[:, bi, :], psc, AF.Copy)
                nc.sync.dma_start(out=x_sb[g * GB : (g + 1) * GB, :D], in_=cs)

            # ============ LSTM tail ============
            # x^T[p, j, b] = x[b, d'=5p+j]
            xT = const.tile([P, NX, BC], BF16)
            nc.sync.dma_start(out=xT, in_=x_sb, transpose=True)

            gates = const.tile([BC, 4 * H], F32)
            for gi in range(G4):
                sl = slice(gi * H, (gi + 1) * H)
                psg = ps_g.tile([BC, H], F32, tag="g")
                nc.tensor.matmul(psg, ones_f, bsum[:, sl], start=True, stop=False)
                for j in range(NX):
                    nc.tensor.matmul(
                        psg, xT[:, j, :], WihT[:, j, sl], start=False, stop=False
                    )
                for j in range(NH):
                    nc.tensor.matmul(
                        psg,
                        hT[:, j, :],
                        WhhT[:, j, sl],
                        start=False,
                        stop=(gi_last := (j == NH - 1)),
                    )
                func = AF.Tanh if gi == 2 else AF.Sigmoid
                nc.scalar.activation(gates[:, sl], psg, func)

            gi_sl = slice(0, H)
            gf_sl = slice(H, 2 * H)
            gg_sl = slice(2 * H, 3 * H)
            go_sl = slice(3 * H, 4 * H)
            fc = lstm.tile([BC, H], F32, tag="fc")
            nc.vector.tensor_mul(fc, gates[:, gf_sl], cn_f)
            ig = lstm.tile([BC, H], F32, tag="ig")
            nc.vector.tensor_mul(ig, gates[:, gi_sl], gates[:, gg_sl])
            c_new = lstm.tile([BC, H], F32, tag="cnew")
            nc.vector.tensor_add(c_new, fc, ig)
            nc.sync.dma_start(out=d_cout[:, :], in_=c_new)
            tc_t = lstm.tile([BC, H], F32, tag="tct")
            nc.scalar.activation(tc_t, c_new, AF.Tanh)
            h_new = lstm.tile([BC, H], F32, tag="hnew")
            nc.vector.tensor_mul(h_new, gates[:, go_sl], tc_t)
            nc.sync.dma_start(out=d_hout[:, :], in_=h_new)

    nc.finalize()
    return nc


def _get_nc():
    global _NC_CACHE
    if _NC_CACHE is None:
        _NC_CACHE = build_nc()
    return _NC_CACHE


def kernel(h, c, batch_H, char_onehots, Wi2h, Wh2h, bh2h, Wscore, W_ih, W_hh, b_ih, b_hh):
    h = np.ascontiguousarray(np.asarray(h, np.float32))
    c = np.ascontiguousarray(np.asarray(c, np.float32))
    batch_H = np.ascontiguousarray(np.asarray(batch_H, np.float32))
    char_onehots = np.ascontiguousarray(np.asarray(char_onehots, np.float32))
    weights = {
        "Wi2h": np.ascontiguousarray(np.asarray(Wi2h, np.float32)),
        "Wh2h": np.ascontiguousarray(np.asarray(Wh2h, np.float32)),
        "bh2h": np.ascontiguousarray(np.asarray(bh2h, np.float32).reshape(1, H)),
        "Wscore": np.ascontiguousarray(np.asarray(Wscore, np.float32).reshape(1, H)),
        "W_ih": np.ascontiguousarray(np.asarray(W_ih, np.float32)),
        "W_hh": np.ascontiguousarray(np.asarray(W_hh, np.float32)),
        "b_ih": np.ascontiguousarray(np.asarray(b_ih, np.float32).reshape(1, 4 * H)),
        "b_hh": np.ascontiguousarray(np.asarray(b_hh, np.float32).reshape(1, 4 * H)),
    }
    nc = _get_nc()
    in_maps = []
    for i in range(NCORES):
        sl = slice(i * BC, (i + 1) * BC)
        m = {
            "h": h[sl],
            "c": c[sl],
            "batch_H": batch_H[sl],
            "char_onehots": char_onehots[sl],
        }
        m.update(weights)
        in_maps.append(m)
    res = run_bass_kernel_spmd(nc, in_maps, list(range(NCORES)), **_run_kwargs)
    _last_result[0] = res
    h_new = np.concatenate([r["h_out"] for r in res.results], axis=0)
    c_new = np.concatenate([r["c_out"] for r in res.results], axis=0)
    alpha = np.concatenate([r["alpha_out"] for r in res.results], axis=0)
    return h_new, c_new, alpha.reshape(B, T, 1)



# revision 9
# speedup vs baseline: 1.7854x; 1.7854x over previous
"""Trainium2 Bass kernel for nn_AttentionCell (additive attention + LSTMCell step).

Data-parallel over 8 NeuronCores: batch B=256 is split into 8 shards of 32;
all parameters are replicated. Each core computes, for its 32 batches:

    proj  = batch_H @ Wi2h^T                      [32, 512, 512]
    p     = h @ Wh2h^T + bh2h                     [32, 512]
    e     = tanh(proj + p[:, None, :]) @ Wscore^T [32, 512]
    alpha = softmax(e, axis=t)                    [32, 512]
    ctx   = sum_t alpha * batch_H                 [32, 512]
    x     = [ctx, char_onehots]                   [32, 608]
    gates = x @ W_ih^T + b_ih + h @ W_hh^T + b_hh [32, 2048]
    i,f,g,o -> c_new = sig(f)*c + sig(i)*tanh(g); h_new = sig(o)*tanh(c_new)

The host converts batch_H / h / weights to bf16 (and pads W_ih to 640 cols)
before the transfer, so the device reads half the bytes and every transpose
is a single xbar DMA-transpose straight out of DRAM: batch_H is read once
naturally (for the context matvec) and once transposed (for the projection
matmul). The h2h bias rides the ScalarE tanh as a per-partition bias; the
projection/e/context/gates all run as bf16 matmuls with f32 PSUM accum.
"""

import sys

if "/opt/trn_rl_repo" not in sys.path:
    sys.path.insert(0, "/opt/trn_rl_repo")

from contextlib import ExitStack

import ml_dtypes
import numpy as np

import concourse.bass as bass
import concourse.tile as tile
from concourse import bacc, mybir
from concourse.bass_utils import run_bass_kernel_spmd

F32 = mybir.dt.float32
BF16 = mybir.dt.bfloat16
AF = mybir.ActivationFunctionType
AX = mybir.AxisListType
NPBF16 = ml_dtypes.bfloat16

B, T, D, H, NE = 256, 512, 512, 512, 96
NCORES = 8
BC = B // NCORES  # 32 batches per core
P = 128
ND, NH, NT = D // P, H // P, T // P  # 4, 4, 4
GB = 8              # batches per softmax group
NGRP = BC // GB     # 4 groups
DX = 640            # padded x width: 512 ctx + 96 onehots + 32 zero pad
NX = DX // P        # 5 contraction chunks for W_ih
G4 = 4              # four 512-wide gate blocks (i, f, g, o)

_NC_CACHE = None
_run_kwargs = {}
_last_result = [None]


def build_nc():
    nc = bacc.Bacc()

    # ---- per-core I/O (host pre-converts dtypes / layouts) ----
    d_hbf = nc.declare_dram_parameter("h_bf", [BC, H], BF16, isOutput=False)
    d_c = nc.declare_dram_parameter("c", [BC, H], F32, isOutput=False)
    d_bh = nc.declare_dram_parameter("batch_H_bf", [BC, T, D], BF16, isOutput=False)
    d_oh = nc.declare_dram_parameter("char_onehots", [BC, NE], F32, isOutput=False)
    d_wi2h = nc.declare_dram_parameter("Wi2h_bf", [H, D], BF16, isOutput=False)
    d_wh2h = nc.declare_dram_parameter("Wh2h_bf", [H, H], BF16, isOutput=False)
    d_bh2hT = nc.declare_dram_parameter("bh2hT", [H, 1], F32, isOutput=False)
    d_wscT = nc.declare_dram_parameter("WscoreT_bf", [H, 1], BF16, isOutput=False)
    d_wih = nc.declare_dram_parameter("W_ih_pad_bf", [4 * H, DX], BF16, isOutput=False)
    d_whh = nc.declare_dram_parameter("W_hh_bf", [4 * H, H], BF16, isOutput=False)
    d_bsum = nc.declare_dram_parameter("bias_sum", [1, 4 * H], F32, isOutput=False)
    d_hout = nc.declare_dram_parameter("h_out", [BC, H], F32, isOutput=True)
    d_cout = nc.declare_dram_parameter("c_out", [BC, H], F32, isOutput=True)
    d_aout = nc.declare_dram_parameter("alpha_out", [BC, T], F32, isOutput=True)

    with tile.TileContext(nc) as tc, ExitStack() as ctx:
        const = ctx.enter_context(tc.tile_pool(name="const", bufs=1))
        lstm = ctx.enter_context(tc.tile_pool(name="lstm", bufs=1))
        stag = ctx.enter_context(tc.tile_pool(name="stag", bufs=1))
        xbp = ctx.enter_context(tc.tile_pool(name="xbp", bufs=12))
        xtp = ctx.enter_context(tc.tile_pool(name="xtp", bufs=4))
        thp = ctx.enter_context(tc.tile_pool(name="thp", bufs=4))
        grp = ctx.enter_context(tc.tile_pool(name="grp", bufs=2))
        ps_proj = ctx.enter_context(tc.tile_pool(name="ps_proj", bufs=4, space="PSUM"))
        ps_vec = ctx.enter_context(tc.tile_pool(name="ps_vec", bufs=2, space="PSUM"))
        ps_misc = ctx.enter_context(tc.tile_pool(name="ps_misc", bufs=2, space="PSUM"))

        # ============ constant prep ============
        ones_f = const.tile([1, BC], F32)
        nc.vector.memset(ones_f, 1.0)

        # weight transposes: one DRAM->SBUF xbar transpose each
        # WT[p, j, h] pairs with XT (same 512-wide transpose interleave)
        WT = const.tile([P, ND, H], BF16)
        nc.sync.dma_start(out=WT, in_=d_wi2h[:, :], transpose=True)
        WhT = const.tile([P, NH, H], BF16)
        nc.sync.dma_start(out=WhT, in_=d_wh2h[:, :], transpose=True)
        WihT = const.tile([P, NX, 4 * H], BF16)
        nc.sync.dma_start(out=WihT, in_=d_wih[:, :], transpose=True)
        WhhT = const.tile([P, NH, 4 * H], BF16)
        nc.sync.dma_start(out=WhhT, in_=d_whh[:, :], transpose=True)
        hT = const.tile([P, NH, BC], BF16)
        nc.sync.dma_start(out=hT, in_=d_hbf[:, :], transpose=True)

        # blocked per-partition vectors (host supplies them column-shaped)
        bhT = const.tile([P, NH, 1], F32)
        nc.gpsimd.dma_start(out=bhT, in_=d_bh2hT.rearrange("(n p) o -> p n o", p=P))
        wT = const.tile([P, NH, 1], BF16)
        nc.gpsimd.dma_start(out=wT, in_=d_wscT.rearrange("(n p) o -> p n o", p=P))

        # prev_hidden_proj^T: pT[:, c', b] = (h @ Wh2h^T + bh2h)[b, h'=128c'+p]
        pT = const.tile([P, NH, BC], F32)
        for ci in range(NH):
            psp = ps_misc.tile([P, BC], F32, tag="misc")
            for j in range(NH):
                nc.tensor.matmul(
                    psp,
                    WhT[:, j, ci * P : (ci + 1) * P],
                    hT[:, j, :],
                    start=(j == 0),
                    stop=(j == NH - 1),
                )
            nc.scalar.activation(pT[:, ci, :], psp, AF.Identity, bias=bhT[:, ci, :])

        bsum = const.tile([1, 4 * H], F32)
        nc.gpsimd.dma_start(out=bsum, in_=d_bsum[:, :])

        # x staging [ctx | onehots | 0-pad] in bf16, ctx rows filled later
        x_sb = const.tile([BC, DX], BF16)
        nc.vector.memset(x_sb[:, D:], 0.0)
        oh_f = lstm.tile([BC, NE], F32, tag="ldoh")
        nc.gpsimd.dma_start(out=oh_f, in_=d_oh[:, :])
        nc.vector.tensor_copy(x_sb[:, D : D + NE], oh_f)

        cn_f = lstm.tile([BC, H], F32, tag="cn")
        nc.gpsimd.dma_start(out=cn_f, in_=d_c[:, :])

        # ============ main loop: groups of 8 batches ============
        xb_tiles = {}
        for g in range(NGRP):
            es = stag.tile([1, GB, T], BF16, tag="es")
            for bi in range(GB):
                b = g * GB + bi
                # transposed load: XT[p, j, t] = batch_H[b, t, d] (d split 4-way)
                xt = xtp.tile([P, ND, T], BF16, tag="xt")
                nc.sync.dma_start(out=xt, in_=d_bh[b], transpose=True)
                # natural load: Xb[p, tcb, d] = batch_H[b, t=128*tcb+p, d]
                xb = xbp.tile([P, NT, D], BF16, tag="xb")
                xb_tiles[b] = xb
                nc.gpsimd.dma_start(
                    out=xb, in_=d_bh[b].rearrange("(n p) d -> p n d", p=P)
                )
                # proj + tanh(. + p_b) per h-tile
                th = thp.tile([P, NH, T], BF16, tag="th")
                for ht in range(NH):
                    psy = ps_proj.tile([P, T], F32, tag="proj")
                    for j in range(ND):
                        nc.tensor.matmul(
                            psy,
                            WT[:, j, ht * P : (ht + 1) * P],
                            xt[:, j, :],
                            start=(j == 0),
                            stop=(j == ND - 1),
                        )
                    nc.scalar.activation(
                        th[:, ht, :], psy, AF.Tanh, bias=pT[:, ht, b : b + 1]
                    )
                # e[b, t] = sum_h Wscore[h] * tanh(...)
                pse = ps_vec.tile([1, T], F32, tag="vec")
                for ht in range(NH):
                    nc.tensor.matmul(
                        pse,
                        wT[:, ht, :],
                        th[:, ht, :],
                        start=(ht == 0),
                        stop=(ht == NH - 1),
                    )
                nc.vector.tensor_copy(es[:, bi, :], pse)

            # gather the group's e rows onto 8 partitions
            E = grp.tile([GB, T], BF16, tag="E")
            nc.gpsimd.dma_start(out=E, in_=es)
            # softmax over t (free dim)
            mx = grp.tile([GB, 1], F32, tag="mx")
            nc.vector.reduce_max(mx, E, axis=AX.X)
            nmx = grp.tile([GB, 1], F32, tag="nmx")
            nc.vector.tensor_scalar_mul(nmx, mx, -1.0)
            ex = grp.tile([GB, T], F32, tag="ex")
            ssum = grp.tile([GB, 1], F32, tag="ssum")
            nc.scalar.activation(ex, E, AF.Exp, bias=nmx, accum_out=ssum)
            rs = grp.tile([GB, 1], F32, tag="rs")
            nc.vector.reciprocal(rs, ssum)
            alpha = grp.tile([GB, T], F32, tag="alpha")
            nc.vector.tensor_scalar_mul(alpha, ex, rs)
            nc.gpsimd.dma_start(out=d_aout[g * GB : (g + 1) * GB, :], in_=alpha)
            # alpha^T tiles (bf16, t-blocked to match Xb's layout)
            alpha_bf = grp.tile([2 * GB, T], BF16, tag="alpha_bf")
            nc.vector.memset(alpha_bf, 0.0)
            nc.vector.tensor_copy(alpha_bf[:GB, :], alpha)
            aT = grp.tile([P, NT, 2 * GB], BF16, tag="aT")
            for tcb in range(NT):
                nc.sync.dma_start(
                    out=aT[:, tcb, :],
                    in_=alpha_bf[:, tcb * P : (tcb + 1) * P],
                    transpose=True,
                )
            # context rows
            cs = stag.tile([1, GB, D], BF16, tag="cs")
            for bi in range(GB):
                b = g * GB + bi
                psc = ps_vec.tile([1, D], F32, tag="vec")
                for tcb in range(NT):
                    nc.tensor.matmul(
                        psc,
                        aT[:, tcb, bi : bi + 1],
                        xb_tiles[b][:, tcb, :],
                        start=(tcb == 0),
                        stop=(tcb == NT - 1),
                    )
                nc.vector.tensor_copy(cs[:, bi, :], psc)
            nc.gpsimd.dma_start(out=x_sb[g * GB : (g + 1) * GB, :D], in_=cs)

        # ============ LSTM tail ============
        # x^T[p, j, b] = x[b, d'=5p+j]  (pairs with WihT's 640-wide transpose)
        xT = const.tile([P, NX, BC], BF16)
        nc.sync.dma_start(out=xT, in_=x_sb, transpose=True)

        gates = const.tile([BC, 4 * H], F32)
        for gi in range(G4):
            sl = slice(gi * H, (gi + 1) * H)
            psg = ps_misc.tile([BC, H], F32, tag="misc")
            nc.tensor.matmul(psg, ones_f, bsum[:, sl], start=True, stop=False)
            for j in range(NX):
                nc.tensor.matmul(
                    psg, xT[:, j, :], WihT[:, j, sl], start=False, stop=False
                )
            for j in range(NH):
                nc.tensor.matmul(
                    psg,
                    hT[:, j, :],
                    WhhT[:, j, sl],
                    start=False,
                    stop=(j == NH - 1),
                )
            func = AF.Tanh if gi == 2 else AF.Sigmoid
            nc.scalar.activation(gates[:, sl], psg, func)

        gi_sl = slice(0, H)
        gf_sl = slice(H, 2 * H)
        gg_sl = slice(2 * H, 3 * H)
        go_sl = slice(3 * H, 4 * H)
        fc = lstm.tile([BC, H], F32, tag="fc")
        nc.vector.tensor_mul(fc, gates[:, gf_sl], cn_f)
        ig = lstm.tile([BC, H], F32, tag="ig")
        nc.vector.tensor_mul(ig, gates[:, gi_sl], gates[:, gg_sl])
        c_new = lstm.tile([BC, H], F32, tag="cnew")
        nc.vector.tensor_add(c_new, fc, ig)
        nc.gpsimd.dma_start(out=d_cout[:, :], in_=c_new)
        tc_t = lstm.tile([BC, H], F32, tag="tct")
        nc.scalar.activation(tc_t, c_new, AF.Tanh)
        h_new = lstm.tile([BC, H], F32, tag="hnew")
        nc.vector.tensor_mul(h_new, gates[:, go_sl], tc_t)
        nc.gpsimd.dma_start(out=d_hout[:, :], in_=h_new)

    nc.finalize()
    return nc


def _get_nc():
    global _NC_CACHE
    if _NC_CACHE is None:
        _NC_CACHE = build_nc()
    return _NC_CACHE


def prep_host_inputs(h, c, batch_H, char_onehots, Wi2h, Wh2h, bh2h, Wscore,
                     W_ih, W_hh, b_ih, b_hh):
    """dtype/layout-only host prep; weights shared across cores."""
    W_ih = np.asarray(W_ih, np.float32)
    wih_pad = np.zeros((4 * H, DX), np.float32)
    wih_pad[:, : D + NE] = W_ih
    weights = {
        "Wi2h_bf": np.asarray(Wi2h, np.float32).astype(NPBF16),
        "Wh2h_bf": np.asarray(Wh2h, np.float32).astype(NPBF16),
        "bh2hT": np.ascontiguousarray(np.asarray(bh2h, np.float32).reshape(H, 1)),
        "WscoreT_bf": np.ascontiguousarray(
            np.asarray(Wscore, np.float32).reshape(H, 1)
        ).astype(NPBF16),
        "W_ih_pad_bf": wih_pad.astype(NPBF16),
        "W_hh_bf": np.asarray(W_hh, np.float32).astype(NPBF16),
        "bias_sum": (
            np.asarray(b_ih, np.float32) + np.asarray(b_hh, np.float32)
        ).reshape(1, 4 * H),
    }
    per_shard = {
        "h_bf": np.asarray(h, np.float32).astype(NPBF16),
        "c": np.ascontiguousarray(np.asarray(c, np.float32)),
        "batch_H_bf": np.asarray(batch_H, np.float32).astype(NPBF16),
        "char_onehots": np.ascontiguousarray(np.asarray(char_onehots, np.float32)),
    }
    return weights, per_shard


def kernel(h, c, batch_H, char_onehots, Wi2h, Wh2h, bh2h, Wscore, W_ih, W_hh, b_ih, b_hh):
    weights, shard = prep_host_inputs(
        h, c, batch_H, char_onehots, Wi2h, Wh2h, bh2h, Wscore, W_ih, W_hh, b_ih, b_hh
    )
    nc = _get_nc()
    in_maps = []
    for i in range(NCORES):
        sl = slice(i * BC, (i + 1) * BC)
        m = {k: np.ascontiguousarray(v[sl]) for k, v in shard.items()}
        m.update(weights)
        in_maps.append(m)
    res = run_bass_kernel_spmd(nc, in_maps, list(range(NCORES)), **_run_kwargs)
    _last_result[0] = res
    h_new = np.concatenate([r["h_out"] for r in res.results], axis=0)
    c_new = np.concatenate([r["c_out"] for r in res.results], axis=0)
    alpha = np.concatenate([r["alpha_out"] for r in res.results], axis=0)
    return h_new, c_new, alpha.reshape(B, T, 1)


# revision 11
# speedup vs baseline: 1.7946x; 1.0052x over previous
"""Trainium2 Bass kernel for nn_AttentionCell (additive attention + LSTMCell step).

Data-parallel over 8 NeuronCores: batch B=256 is split into 8 shards of 32;
all parameters are replicated. Each core computes, for its 32 batches:

    proj  = batch_H @ Wi2h^T                      [32, 512, 512]
    p     = h @ Wh2h^T + bh2h                     [32, 512]
    e     = tanh(proj + p[:, None, :]) @ Wscore^T [32, 512]
    alpha = softmax(e, axis=t)                    [32, 512]
    ctx   = sum_t alpha * batch_H                 [32, 512]
    x     = [ctx, char_onehots]                   [32, 608]
    gates = x @ W_ih^T + b_ih + h @ W_hh^T + b_hh [32, 2048]
    i,f,g,o -> c_new = sig(f)*c + sig(i)*tanh(g); h_new = sig(o)*tanh(c_new)

The host converts batch_H / h / weights to bf16 (and pads W_ih to 640 cols)
before the transfer, so the device reads half the bytes and every transpose
is a single xbar DMA-transpose straight out of DRAM: batch_H is read once
naturally (for the context matvec) and once transposed (for the projection
matmul). The h2h bias rides the ScalarE tanh as a per-partition bias; the
projection/e/context/gates all run as bf16 matmuls with f32 PSUM accum.
"""

import sys

if "/opt/trn_rl_repo" not in sys.path:
    sys.path.insert(0, "/opt/trn_rl_repo")

from contextlib import ExitStack

import ml_dtypes
import numpy as np

import concourse.bass as bass
import concourse.tile as tile
from concourse import bacc, mybir
from concourse.bass_utils import run_bass_kernel_spmd

F32 = mybir.dt.float32
BF16 = mybir.dt.bfloat16
AF = mybir.ActivationFunctionType
AX = mybir.AxisListType
NPBF16 = ml_dtypes.bfloat16

B, T, D, H, NE = 256, 512, 512, 512, 96
NCORES = 8
BC = B // NCORES  # 32 batches per core
P = 128
ND, NH, NT = D // P, H // P, T // P  # 4, 4, 4
GB = 8              # batches per softmax group
NGRP = BC // GB     # 4 groups
DX = 640            # padded x width: 512 ctx + 96 onehots + 32 zero pad
NX = DX // P        # 5 contraction chunks for W_ih
G4 = 4              # four 512-wide gate blocks (i, f, g, o)

_NC_CACHE = None
_run_kwargs = {}
_last_result = [None]


def build_nc():
    nc = bacc.Bacc()

    # ---- per-core I/O (host pre-converts dtypes / layouts) ----
    d_hbf = nc.declare_dram_parameter("h_bf", [BC, H], BF16, isOutput=False)
    d_c = nc.declare_dram_parameter("c", [BC, H], F32, isOutput=False)
    d_bh = nc.declare_dram_parameter("batch_H_bf", [BC, T, D], BF16, isOutput=False)
    d_oh = nc.declare_dram_parameter("char_onehots", [BC, NE], F32, isOutput=False)
    d_wi2h = nc.declare_dram_parameter("Wi2h_bf", [H, D], BF16, isOutput=False)
    d_wh2h = nc.declare_dram_parameter("Wh2h_bf", [H, H], BF16, isOutput=False)
    d_bh2hT = nc.declare_dram_parameter("bh2hT", [H, 1], F32, isOutput=False)
    d_wscT = nc.declare_dram_parameter("WscoreT_bf", [H, 1], BF16, isOutput=False)
    d_wih = nc.declare_dram_parameter("W_ih_pad_bf", [4 * H, DX], BF16, isOutput=False)
    d_whh = nc.declare_dram_parameter("W_hh_bf", [4 * H, H], BF16, isOutput=False)
    d_bsum = nc.declare_dram_parameter("bias_sum", [1, 4 * H], F32, isOutput=False)
    d_hout = nc.declare_dram_parameter("h_out", [BC, H], F32, isOutput=True)
    d_cout = nc.declare_dram_parameter("c_out", [BC, H], F32, isOutput=True)
    d_aout = nc.declare_dram_parameter("alpha_out", [BC, T], F32, isOutput=True)

    with tile.TileContext(nc) as tc, ExitStack() as ctx:
        const = ctx.enter_context(tc.tile_pool(name="const", bufs=1))
        lstm = ctx.enter_context(tc.tile_pool(name="lstm", bufs=1))
        stag = ctx.enter_context(tc.tile_pool(name="stag", bufs=1))
        xbp = ctx.enter_context(tc.tile_pool(name="xbp", bufs=12))
        xtp = ctx.enter_context(tc.tile_pool(name="xtp", bufs=4))
        thp = ctx.enter_context(tc.tile_pool(name="thp", bufs=4))
        grp = ctx.enter_context(tc.tile_pool(name="grp", bufs=2))
        ps_proj = ctx.enter_context(tc.tile_pool(name="ps_proj", bufs=4, space="PSUM"))
        ps_vec = ctx.enter_context(tc.tile_pool(name="ps_vec", bufs=2, space="PSUM"))
        ps_misc = ctx.enter_context(tc.tile_pool(name="ps_misc", bufs=2, space="PSUM"))

        # ============ constant prep ============
        ones_f = const.tile([1, BC], F32)
        nc.vector.memset(ones_f, 1.0)

        # weight transposes: one DRAM->SBUF xbar transpose each
        # WT[p, j, h] pairs with XT (same 512-wide transpose interleave)
        WT = const.tile([P, ND, H], BF16)
        nc.sync.dma_start(out=WT, in_=d_wi2h[:, :], transpose=True)
        WhT = const.tile([P, NH, H], BF16)
        nc.sync.dma_start(out=WhT, in_=d_wh2h[:, :], transpose=True)
        WihT = const.tile([P, NX, 4 * H], BF16)
        nc.sync.dma_start(out=WihT, in_=d_wih[:, :], transpose=True)
        WhhT = const.tile([P, NH, 4 * H], BF16)
        nc.sync.dma_start(out=WhhT, in_=d_whh[:, :], transpose=True)
        hT = const.tile([P, NH, BC], BF16)
        nc.sync.dma_start(out=hT, in_=d_hbf[:, :], transpose=True)

        # blocked per-partition vectors (host supplies them column-shaped)
        bhT = const.tile([P, NH, 1], F32)
        nc.gpsimd.dma_start(out=bhT, in_=d_bh2hT.rearrange("(n p) o -> p n o", p=P))
        wT = const.tile([P, NH, 1], BF16)
        nc.gpsimd.dma_start(out=wT, in_=d_wscT.rearrange("(n p) o -> p n o", p=P))

        # prev_hidden_proj^T: pT[:, c', b] = (h @ Wh2h^T + bh2h)[b, h'=128c'+p]
        pT = const.tile([P, NH, BC], F32)
        for ci in range(NH):
            psp = ps_misc.tile([P, BC], F32, tag="misc")
            for j in range(NH):
                nc.tensor.matmul(
                    psp,
                    WhT[:, j, ci * P : (ci + 1) * P],
                    hT[:, j, :],
                    start=(j == 0),
                    stop=(j == NH - 1),
                )
            nc.scalar.activation(pT[:, ci, :], psp, AF.Identity, bias=bhT[:, ci, :])

        bsum = const.tile([1, 4 * H], F32)
        nc.gpsimd.dma_start(out=bsum, in_=d_bsum[:, :])

        # x staging [ctx | onehots | 0-pad] in bf16, ctx rows filled later
        x_sb = const.tile([BC, DX], BF16)
        nc.vector.memset(x_sb[:, D:], 0.0)
        oh_f = lstm.tile([BC, NE], F32, tag="ldoh")
        nc.gpsimd.dma_start(out=oh_f, in_=d_oh[:, :])
        nc.vector.tensor_copy(x_sb[:, D : D + NE], oh_f)

        cn_f = lstm.tile([BC, H], F32, tag="cn")
        nc.gpsimd.dma_start(out=cn_f, in_=d_c[:, :])

        # ============ main loop: groups of 8 batches ============
        xb_tiles = {}
        for g in range(NGRP):
            es = stag.tile([1, GB, T], BF16, tag="es")
            for bi in range(GB):
                b = g * GB + bi
                # transposed load: xt[p, j, t] = batch_H[b, t, d] (d split 4-way)
                xt = xtp.tile([P, ND, T], BF16, tag="xt")
                nc.sync.dma_start(out=xt, in_=d_bh[b], transpose=True)
                # natural load: Xb[p, tcb, d] = batch_H[b, t=128*tcb+p, d]
                xb = xbp.tile([P, NT, D], BF16, tag="xb")
                xb_tiles[b] = xb
                nc.gpsimd.dma_start(
                    out=xb, in_=d_bh[b].rearrange("(n p) d -> p n d", p=P)
                )
                # proj + tanh(. + p_b) per h-tile
                th = thp.tile([P, NH, T], BF16, tag="th")
                for ht in range(NH):
                    psy = ps_proj.tile([P, T], F32, tag="proj")
                    for j in range(ND):
                        nc.tensor.matmul(
                            psy,
                            WT[:, j, ht * P : (ht + 1) * P],
                            xt[:, j, :],
                            start=(j == 0),
                            stop=(j == ND - 1),
                        )
                    nc.scalar.activation(
                        th[:, ht, :], psy, AF.Tanh, bias=pT[:, ht, b : b + 1]
                    )
                # e[b, t] = sum_h Wscore[h] * tanh(...)
                pse = ps_vec.tile([1, T], F32, tag="vec")
                for ht in range(NH):
                    nc.tensor.matmul(
                        pse,
                        wT[:, ht, :],
                        th[:, ht, :],
                        start=(ht == 0),
                        stop=(ht == NH - 1),
                    )
                nc.vector.tensor_copy(es[:, bi, :], pse)

            # gather the group's e rows onto 8 partitions
            E = grp.tile([GB, T], BF16, tag="E")
            nc.gpsimd.dma_start(out=E, in_=es)
            # softmax over t (free dim)
            mx = grp.tile([GB, 1], F32, tag="mx")
            nc.vector.reduce_max(mx, E, axis=AX.X)
            nmx = grp.tile([GB, 1], F32, tag="nmx")
            nc.vector.tensor_scalar_mul(nmx, mx, -1.0)
            ex = grp.tile([GB, T], F32, tag="ex")
            ssum = grp.tile([GB, 1], F32, tag="ssum")
            nc.scalar.activation(ex, E, AF.Exp, bias=nmx, accum_out=ssum)
            rs = grp.tile([GB, 1], F32, tag="rs")
            nc.vector.reciprocal(rs, ssum)
            alpha = grp.tile([GB, T], F32, tag="alpha")
            nc.vector.tensor_scalar_mul(alpha, ex, rs)
            nc.gpsimd.dma_start(out=d_aout[g * GB : (g + 1) * GB, :], in_=alpha)
            # alpha^T tiles (bf16, t-blocked to match Xb's layout)
            alpha_bf = grp.tile([2 * GB, T], BF16, tag="alpha_bf")
            nc.vector.memset(alpha_bf, 0.0)
            nc.vector.tensor_copy(alpha_bf[:GB, :], alpha)
            aT = grp.tile([P, NT, 2 * GB], BF16, tag="aT")
            for tcb in range(NT):
                nc.sync.dma_start(
                    out=aT[:, tcb, :],
                    in_=alpha_bf[:, tcb * P : (tcb + 1) * P],
                    transpose=True,
                )
            # context rows
            cs = stag.tile([1, GB, D], BF16, tag="cs")
            for bi in range(GB):
                b = g * GB + bi
                psc = ps_vec.tile([1, D], F32, tag="vec")
                for tcb in range(NT):
                    nc.tensor.matmul(
                        psc,
                        aT[:, tcb, bi : bi + 1],
                        xb_tiles[b][:, tcb, :],
                        start=(tcb == 0),
                        stop=(tcb == NT - 1),
                    )
                nc.vector.tensor_copy(cs[:, bi, :], psc)
            nc.gpsimd.dma_start(out=x_sb[g * GB : (g + 1) * GB, :D], in_=cs)

        # ============ LSTM tail ============
        # x^T[p, j, b] = x[b, d'=5p+j]  (pairs with WihT's 640-wide transpose)
        xT = const.tile([P, NX, BC], BF16)
        nc.sync.dma_start(out=xT, in_=x_sb, transpose=True)

        gates = const.tile([BC, 4 * H], F32)
        for gi in range(G4):
            sl = slice(gi * H, (gi + 1) * H)
            psg = ps_misc.tile([BC, H], F32, tag="misc")
            nc.tensor.matmul(psg, ones_f, bsum[:, sl], start=True, stop=False)
            for j in range(NX):
                nc.tensor.matmul(
                    psg, xT[:, j, :], WihT[:, j, sl], start=False, stop=False
                )
            for j in range(NH):
                nc.tensor.matmul(
                    psg,
                    hT[:, j, :],
                    WhhT[:, j, sl],
                    start=False,
                    stop=(j == NH - 1),
                )
            func = AF.Tanh if gi == 2 else AF.Sigmoid
            nc.scalar.activation(gates[:, sl], psg, func)

        gi_sl = slice(0, H)
        gf_sl = slice(H, 2 * H)
        gg_sl = slice(2 * H, 3 * H)
        go_sl = slice(3 * H, 4 * H)
        fc = lstm.tile([BC, H], F32, tag="fc")
        nc.vector.tensor_mul(fc, gates[:, gf_sl], cn_f)
        ig = lstm.tile([BC, H], F32, tag="ig")
        nc.vector.tensor_mul(ig, gates[:, gi_sl], gates[:, gg_sl])
        c_new = lstm.tile([BC, H], F32, tag="cnew")
        nc.vector.tensor_add(c_new, fc, ig)
        nc.gpsimd.dma_start(out=d_cout[:, :], in_=c_new)
        tc_t = lstm.tile([BC, H], F32, tag="tct")
        nc.scalar.activation(tc_t, c_new, AF.Tanh)
        h_new = lstm.tile([BC, H], F32, tag="hnew")
        nc.vector.tensor_mul(h_new, gates[:, go_sl], tc_t)
        nc.gpsimd.dma_start(out=d_hout[:, :], in_=h_new)

    nc.finalize()
    return nc


def _get_nc():
    global _NC_CACHE
    if _NC_CACHE is None:
        _NC_CACHE = build_nc()
    return _NC_CACHE


def prep_host_inputs(h, c, batch_H, char_onehots, Wi2h, Wh2h, bh2h, Wscore,
                     W_ih, W_hh, b_ih, b_hh):
    """dtype/layout-only host prep; weights shared across cores."""
    W_ih = np.asarray(W_ih, np.float32)
    wih_pad = np.zeros((4 * H, DX), np.float32)
    wih_pad[:, : D + NE] = W_ih
    weights = {
        "Wi2h_bf": np.asarray(Wi2h, np.float32).astype(NPBF16),
        "Wh2h_bf": np.asarray(Wh2h, np.float32).astype(NPBF16),
        "bh2hT": np.ascontiguousarray(np.asarray(bh2h, np.float32).reshape(H, 1)),
        "WscoreT_bf": np.ascontiguousarray(
            np.asarray(Wscore, np.float32).reshape(H, 1)
        ).astype(NPBF16),
        "W_ih_pad_bf": wih_pad.astype(NPBF16),
        "W_hh_bf": np.asarray(W_hh, np.float32).astype(NPBF16),
        "bias_sum": (
            np.asarray(b_ih, np.float32) + np.asarray(b_hh, np.float32)
        ).reshape(1, 4 * H),
    }
    per_shard = {
        "h_bf": np.asarray(h, np.float32).astype(NPBF16),
        "c": np.ascontiguousarray(np.asarray(c, np.float32)),
        "batch_H_bf": np.asarray(batch_H, np.float32).astype(NPBF16),
        "char_onehots": np.ascontiguousarray(np.asarray(char_onehots, np.float32)),
    }
    return weights, per_shard


def kernel(h, c, batch_H, char_onehots, Wi2h, Wh2h, bh2h, Wscore, W_ih, W_hh, b_ih, b_hh):
    weights, shard = prep_host_inputs(
        h, c, batch_H, char_onehots, Wi2h, Wh2h, bh2h, Wscore, W_ih, W_hh, b_ih, b_hh
    )
    nc = _get_nc()
    in_maps = []
    for i in range(NCORES):
        sl = slice(i * BC, (i + 1) * BC)
        m = {k: np.ascontiguousarray(v[sl]) for k, v in shard.items()}
        m.update(weights)
        in_maps.append(m)
    res = run_bass_kernel_spmd(nc, in_maps, list(range(NCORES)), **_run_kwargs)
    _last_result[0] = res
    h_new = np.concatenate([r["h_out"] for r in res.results], axis=0)
    c_new = np.concatenate([r["c_out"] for r in res.results], axis=0)
    alpha = np.concatenate([r["alpha_out"] for r in res.results], axis=0)
    return h_new, c_new, alpha.reshape(B, T, 1)
